# revision 14
# baseline (speedup 1.0000x reference)
"""HQLinear (VQ codebook linear) on 8 Trainium2 NeuronCores.

Strategy (column-parallel, per the sharding hint):
- Host: dequantize w = codebook[indices].reshape(O, I) * scales (scales folded
  into w), pre-transpose x -> xT [I, T] and w -> wT [I, O], cast to fp16.
- Shard wT along out_features across 8 cores (512 outs each); x replicated.
- Device per core: outT_shard[o, t] = wT_shard.T @ xT via fp16 matmuls
  (fp32 PSUM accumulate), K accumulated in PSUM in groups of KG k-tiles,
  group partials DVE-accumulated into an SBUF fp32 accumulator so x is
  streamed from HBM exactly once.
- Host: stack shards -> [O, T], transpose -> [T, O] -> reshape (2, 2048, O).
"""
import numpy as np

import concourse.mybir as mybir
import concourse.tile as tile
from concourse import bacc
from concourse.bass_utils import run_bass_kernel_spmd

B, S, IN_F, OUT_F, VEC = 2, 2048, 4096, 4096, 8
T = B * S                      # 4096 tokens
NCORES = 8
OSH = OUT_F // NCORES          # 512 outs per core
KT = IN_F // 128               # 32 k-tiles
TCH = T // 512                 # 8 token chunks
NOT = OSH // 128               # 4 o-tiles per core

F32 = mybir.dt.float32

_CFG = {
    # dt, np_dt, KG (k-tiles per PSUM group), x-slab bufs, w bufs
    "f16": (mybir.dt.float16, np.float16, 8, 24, 18),
    "bf16": (mybir.dt.bfloat16, None, 8, 12, 18),
    "f32r": (mybir.dt.float32r, np.float32, 4, 6, 8),
}

_BUILD_CACHE = {}


def _build(dt_key):
    if dt_key in _BUILD_CACHE:
        return _BUILD_CACHE[dt_key]
    DT, _, KG, XBUFS, WBUFS = _CFG[dt_key]
    NKG = KT // KG
    nc = bacc.Bacc("TRN2", target_bir_lowering=False, debug=False, num_devices=NCORES)
    xT = nc.dram_tensor("xT", [IN_F, T], DT, kind="ExternalInput")
    wT = nc.dram_tensor("wT", [IN_F, OSH], DT, kind="ExternalInput")
    outT = nc.dram_tensor("outT", [OSH, T], F32, kind="ExternalOutput")

    with tile.TileContext(nc) as tc:
        with (
            tc.tile_pool(name="accp", bufs=1) as accp,
            tc.tile_pool(name="xp", bufs=XBUFS) as xp,
            tc.tile_pool(name="wp", bufs=WBUFS) as wp,
            tc.tile_pool(name="psum", bufs=8, space="PSUM") as psp,
        ):
            acc = accp.tile([128, NOT * T], F32)  # 8 MB accumulator

            HT = T // 2  # x slab half width
            for kg in range(NKG):
                xts = []
                wts = []
                for j in range(KG):
                    k = kg * KG + j
                    wt = wp.tile([128, OSH], DT, tag="wslab", name=f"w_{k}")
                    nc.sync.dma_start(out=wt[:], in_=wT[k * 128:(k + 1) * 128, :])
                    wts.append(wt)
                    halves = []
                    for h in range(2):
                        xt = xp.tile([128, HT], DT, tag="xslab", name=f"x_{k}_{h}")
                        nc.sync.dma_start(
                            out=xt[:], in_=xT[k * 128:(k + 1) * 128, h * HT:(h + 1) * HT]
                        )
                        halves.append(xt)
                    xts.append(halves)
                for ot in range(NOT):
                    pss = [
                        psp.tile([128, 512], F32, tag="mmps", name=f"ps_{kg}_{ot}_{i}")
                        for i in range(TCH)
                    ]
                    for j in range(KG):
                        for tch in range(TCH):
                            h, u = divmod(tch, TCH // 2)
                            nc.tensor.matmul(
                                out=pss[tch][:],
                                lhsT=wts[j][:, ot * 128:(ot + 1) * 128],
                                rhs=xts[j][h][:, u * 512:(u + 1) * 512],
                                start=(j == 0),
                                stop=(j == KG - 1),
                            )
                    for tch in range(TCH):
                        dst = acc[:, (ot * TCH + tch) * 512:(ot * TCH + tch + 1) * 512]
                        if kg == 0:
                            nc.vector.tensor_copy(out=dst, in_=pss[tch][:])
                        else:
                            nc.vector.tensor_add(out=dst, in0=dst, in1=pss[tch][:])
                        if kg == NKG - 1 and tch % 2 == 1:
                            lo = (ot * TCH + tch - 1) * 512
                            nc.sync.dma_start(
                                out=outT[ot * 128:(ot + 1) * 128,
                                         (tch - 1) * 512:(tch + 1) * 512],
                                in_=acc[:, lo:lo + 1024],
                            )
    nc.compile()
    _BUILD_CACHE[dt_key] = nc
    return nc


def _np_cast(a, dt_key):
    if dt_key == "bf16":
        import ml_dtypes
        return a.astype(ml_dtypes.bfloat16)
    return a.astype(_CFG[dt_key][1])


def kernel(x, indices, codebook, scales, _want_trace=False, _dt="f16"):
    x = np.asarray(x, dtype=np.float32)
    indices = np.asarray(indices, dtype=np.int32)
    codebook = np.asarray(codebook, dtype=np.float32)
    scales = np.asarray(scales, dtype=np.float32)

    # host dequant + layouts (scales folded into w)
    w = codebook[indices].reshape(OUT_F, IN_F) * scales          # [o, i]
    xT = _np_cast(np.ascontiguousarray(x.reshape(T, IN_F).T), _dt)   # [i, t]
    wT = _np_cast(np.ascontiguousarray(w.T), _dt)                    # [i, o]

    nc = _build(_dt)
    in_maps = [
        {"xT": xT, "wT": np.ascontiguousarray(wT[:, c * OSH:(c + 1) * OSH])}
        for c in range(NCORES)
    ]
    res = run_bass_kernel_spmd(
        nc, in_maps, core_ids=list(range(NCORES)), trace=_want_trace
    )
    out_o_t = np.concatenate([res.results[c]["outT"] for c in range(NCORES)], axis=0)
    out = np.ascontiguousarray(out_o_t.T).reshape(B, S, OUT_F)
    if _want_trace:
        kernel._last_exec_time_ns = res.exec_time_ns
        kernel._last_trace = res.instructions_and_trace
    return out
